# revision 31
# baseline (speedup 1.0000x reference)
"""Trainium2 Bass kernel for CapsuleLayer (dynamic routing) on 8 NeuronCores.

Problem: x[32,1152,64], W[1152,32,64,64], bias[1,1152,32,1] (zeros) ->
         out[32,32,64]
  inputs_hat = einsum('bip,icpq->bicq', x, W)
  3 rounds of routing (softmax over capsule axis, squash, agreement update).

Sharding: input-capsule axis i=1152 split over 8 cores (144 each).
  - W read exactly once per core (75.5 MB), ih materialized to HBM per core.
  - softmax over output capsules c is core-local; only the [32,32,64]
    pre-squash sum is AllReduced per routing round (split into two
    AllReduces so the first overlaps the tail of the round's compute).

Phase 1 per core: 72 block-diagonal matmul pairs (2 i's packed -> K=128)
  compute ih[b,i,c,q]. x and W are split hi/lo into bf16 on the host;
  ih = x_hi@W_hi + x_hi@W_lo + x_lo@W_hi runs at full PE rate with ~1e-5
  relative error. Extra lhsT columns accumulate s0 = sum_i ih / 32 in PSUM
  (bias==0 so round-0 routing coefficients are uniform 1/32). ih is stored
  to HBM in bf16 (round-1 logit pass) and fp32 (final round).

Routing rounds r=1,2: one fused pass per 4-i sweep over ih:
  logits += <ih, v> (DVE mult + reduce over q), softmax over c via exp on
  ScalarE broadcast to the (c,q) grid, s += exp * ih with the 1/Z softmax
  denominator folded into the selector matmul weights (PSUM accumulation).
  Round 1 runs in bf16 (it only steers coefficients); the final round runs
  fp32 end-to-end from the fp32 ih copy.
"""

import os
import sys

import numpy as np

for _p in (
    "/opt/trn_rl_repo",
    "/root/.axon_site",
    "/root/.axon_site/_ro/trn_rl_repo",
    "/root/.axon_site/_ro/pypackages",
):
    if os.path.isdir(_p) and _p not in sys.path:
        sys.path.append(_p)

import ml_dtypes
import concourse.bacc as bacc
import concourse.mybir as mybir
import concourse.tile as tile
from concourse.bass_utils import run_bass_kernel_spmd

F32 = mybir.dt.float32
BF16 = mybir.dt.bfloat16
AF = mybir.ActivationFunctionType
AX = mybir.AxisListType
ALU = mybir.AluOpType
BF = ml_dtypes.bfloat16

B, I, P, C, Q = 32, 1152, 64, 32, 64
N_CORES = 8
IL = I // N_CORES          # 144 input capsules per core
NPAIR = IL // 2            # 72 matmul pairs
NSWEEP = IL // 4           # 36 routing sweeps (4 i's x 32 b = 128 partitions)
CQ = C * Q                 # 2048
NUM_ROUTING = 3
SPLIT_PAIR = 56            # s0 partial AllReduced after this many pairs
SPLIT_SWEEP = 30           # routing partial AllReduced after this many sweeps

CONFIG = {
    "r2_f32": False,          # final round weighted sum in fp32 (else bf16)
    "p1_single": True,        # phase-1 matmul = x_hi @ W_hi only (skip the
                              # hi/lo correction terms: ih is consumed through
                              # its bf16 copy anyway)
    "trace": False,           # profile the run (exec_time_ns); needs ntff hook
    "trace_cores": None,      # None -> core 0 only
}

_compiled = None
_compiled_cfg = None


def _build_kernel():
    """Build + compile the SPMD Bass module (identical program on 8 cores)."""
    nc = bacc.Bacc("TRN2", target_bir_lowering=False, debug=False,
                   num_devices=N_CORES)

    p1s = CONFIG["p1_single"]
    lstride = 96 if p1s else 192
    # lhsT_all[p, t*stride + (0:96)] = hi half, (96:192) = lo half for pair t
    lall_d = nc.dram_tensor("lhsT", [128, NPAIR * lstride], BF16,
                            kind="ExternalInput")
    # w2[t, p, 0:2048] = hi, [t, p, 2048:4096] = lo (lo absent if p1_single)
    w_d = nc.dram_tensor("w_rhs", [NPAIR, 128, CQ if p1s else 2 * CQ], BF16,
                         kind="ExternalInput")
    sel_d = nc.dram_tensor("sel", [128, 32], F32, kind="ExternalInput")
    out_d = nc.dram_tensor("out", [B, CQ], F32, kind="ExternalOutput")

    r2f = CONFIG["r2_f32"]
    assert not (r2f and CONFIG["p1_single"]), \
        "r2_f32 needs the fp32 ih copy from the multi-term phase 1"
    ih_d = nc.dram_tensor("ih_buf", [NPAIR, 64, CQ], F32) if r2f else None
    ihb_d = nc.dram_tensor("ihb_buf", [NPAIR, 64, CQ], BF16)

    rgroups = [list(range(N_CORES))]

    with tile.TileContext(nc) as tc:
        with (
            tc.tile_pool(name="lall", bufs=1) as lall_pool,
            tc.tile_pool(name="w", bufs=3) as w_pool,
            tc.tile_pool(name="cp", bufs=2) as cp_pool,
            tc.tile_pool(name="ih", bufs=4) as ih_pool,
            tc.tile_pool(name="ihw", bufs=2) as ihw_pool,
            tc.tile_pool(name="prod", bufs=3) as prod_pool,
            tc.tile_pool(name="wt", bufs=3) as wt_pool,
            tc.tile_pool(name="v4", bufs=1) as v4_pool,
            tc.tile_pool(name="small", bufs=3) as small_pool,
            tc.tile_pool(name="acc", bufs=1) as acc_pool,
            tc.tile_pool(name="sv", bufs=1) as sv_pool,
            tc.tile_pool(name="psA", bufs=2, space="PSUM") as psA,
            tc.tile_pool(name="psB", bufs=1, space="PSUM") as psB,
            tc.tile_pool(name="dram", bufs=2, space="DRAM") as dram_pool,
        ):
            sel_t = small_pool.tile([128, 32], F32, tag="sel")
            nc.sync.dma_start(sel_t[:], sel_d[:])
            sel_bf = small_pool.tile([128, 32], BF16, tag="selbf")
            nc.vector.tensor_copy(sel_bf[:], sel_t[:])

            b_acc = acc_pool.tile([128, NSWEEP * 32], F32, tag="bacc")
            nc.vector.memset(b_acc[:], 0.0)

            lall = lall_pool.tile([128, NPAIR * lstride], BF16, tag="lall")
            nc.sync.dma_start(lall[:], lall_d[:])

            def flush_and_allreduce(s_ps, tag):
                """PSUM partial -> SBUF -> DRAM -> AllReduce. Returns the
                collective's DRAM output tile."""
                f_sb = sv_pool.tile([32, CQ], F32, tag="s_sb")
                nc.scalar.copy(f_sb[:], s_ps[:])
                a_in = dram_pool.tile([32, CQ], F32, tag="ar_in")
                a_out = dram_pool.tile([32, CQ], F32, tag="ar_out")
                nc.gpsimd.dma_start(a_in[:], f_sb[:])
                nc.gpsimd.collective_compute(
                    "AllReduce", ALU.add,
                    ins=[a_in[:].opt()], outs=[a_out[:].opt()],
                    replica_groups=rgroups,
                )
                return a_out

            # ---------------- Phase 1: ih = x @ W, s0 = sum_i ih / 32 -------
            s_ps = psB.tile([32, CQ], F32, tag="sacc")
            ar_handles = []
            for t in range(NPAIR):
                if t == SPLIT_PAIR:
                    ar_handles.append(flush_and_allreduce(s_ps, "s0a"))
                    s_ps = psB.tile([32, CQ], F32, tag="sacc")
                first, last_t = (t == 0 or t == SPLIT_PAIR), \
                    (t == SPLIT_PAIR - 1 or t == NPAIR - 1)
                lhi = lall[:, t * lstride:t * lstride + 96]
                wt_ = w_pool.tile([128, CQ if p1s else 2 * CQ], BF16)
                nc.sync.dma_start(wt_[:], w_d[t])
                cp = None
                if not p1s:
                    llo = lall[:, t * lstride + 96:t * lstride + 192]
                    cp = cp_pool.tile([64, CQ], F32, tag="cp32")
                cpb = cp_pool.tile([64, CQ], BF16, tag="cpb")
                for ch in range(2):
                    ps = psA.tile([64, 1024], F32)
                    for h in range(2):
                        o = ps[:, 512 * h:512 * (h + 1)]
                        whi = wt_[:, 1024 * ch + 512 * h:1024 * ch + 512 * (h + 1)]
                        if p1s:
                            nc.tensor.matmul(o, lhi[:, 0:64], whi,
                                             start=True, stop=True)
                        else:
                            wlo = wt_[:, CQ + 1024 * ch + 512 * h:
                                      CQ + 1024 * ch + 512 * (h + 1)]
                            nc.tensor.matmul(o, lhi[:, 0:64], whi,
                                             start=True, stop=False)
                            nc.tensor.matmul(o, lhi[:, 0:64], wlo,
                                             start=False, stop=False)
                            nc.tensor.matmul(o, llo[:, 0:64], whi,
                                             start=False, stop=True)
                        nc.tensor.matmul(
                            s_ps[:, 1024 * ch + 512 * h:1024 * ch + 512 * (h + 1)],
                            lhi[:, 64:96], whi,
                            start=first, stop=last_t,
                        )
                    if p1s:
                        # alternate ACT/DVE so neither becomes the pipeline
                        # stage bound (DVE is otherwise idle in phase 1)
                        if ch == 0:
                            nc.scalar.copy(cpb[:, 0:1024], ps[:])
                        else:
                            nc.vector.tensor_copy(cpb[:, 1024:2048], ps[:])
                    else:
                        nc.scalar.copy(cp[:, 1024 * ch:1024 * (ch + 1)], ps[:])
                if not p1s:
                    nc.vector.tensor_copy(cpb[:], cp[:])
                    if r2f:
                        nc.sync.dma_start(ih_d[t], cp[:])
                nc.sync.dma_start(ihb_d[t], cpb[:])
            ar_handles.append(flush_and_allreduce(s_ps, "s0b"))

            # ---------------- Routing rounds -------------------------------
            # The [32, b*(c,q)] sum is reshaped to [128, 512] (partition =
            # (b, cq-quarter)) so the squash math runs on all 128 lanes.
            CQ4 = CQ // 4   # 512
            C4 = C // 4     # 8 capsules per partition-quarter
            for r in range(1, NUM_ROUTING + 1):
                # combine the two AllReduce halves, then squash -> v
                pa = sv_pool.tile([128, CQ4], F32, tag="pa")
                pb = sv_pool.tile([128, CQ4], F32, tag="pb")
                nc.gpsimd.dma_start(pa[:], ar_handles[0][:])
                nc.gpsimd.dma_start(pb[:], ar_handles[1][:])
                S_sb = sv_pool.tile([128, CQ4], F32, tag="S_sb")
                nc.vector.tensor_add(S_sb[:], pa[:], pb[:])

                # squash: v = S * sqrt(sq)/(1+sq),  sq = sum_q S^2
                S3 = S_sb[:].rearrange("b (c q) -> b c q", q=Q)
                sq = small_pool.tile([128, C4], F32, tag="sq")
                sqr = sv_pool.tile([128, CQ4], F32, tag="sqr")
                nc.vector.tensor_mul(sqr[:], S_sb[:], S_sb[:])
                nc.vector.reduce_sum(
                    sq[:], sqr[:].rearrange("b (c q) -> b c q", q=Q),
                    axis=AX.X)
                rt = small_pool.tile([128, C4], F32, tag="rt")
                nc.scalar.sqrt(rt[:], sq[:])
                onep = small_pool.tile([128, C4], F32, tag="onep")
                nc.vector.tensor_scalar_add(onep[:], sq[:], 1.0)
                rden = small_pool.tile([128, C4], F32, tag="rden")
                nc.vector.reciprocal(rden[:], onep[:])
                scale = small_pool.tile([128, C4], F32, tag="scale")
                nc.vector.tensor_mul(scale[:], rt[:], rden[:])
                # v overwrites S_sb in place
                nc.vector.tensor_mul(
                    S3, S3, scale[:].unsqueeze(-1).broadcast_to((128, C4, Q)))
                v_t = S_sb

                if r == NUM_ROUTING:
                    nc.sync.dma_start(out_d[:], v_t[:])
                    break

                last = (r == NUM_ROUTING - 1) and r2f
                SDT = F32 if last else BF16   # sweep compute dtype

                # broadcast v across the 4 i-slots of the 128 partitions
                # (DRAM bounce; the stride-0 broadcast read is legal there)
                v_c = sv_pool.tile([128, CQ4], SDT, tag="v_c")
                nc.vector.tensor_copy(v_c[:], v_t[:])
                vb = dram_pool.tile([32, CQ], SDT, tag="vb")
                nc.gpsimd.dma_start(vb[:], v_c[:])
                v4 = v4_pool.tile([128, CQ], SDT, tag="v4")
                nc.gpsimd.dma_start(
                    v4[:], vb[:].unsqueeze(0).broadcast_to((4, 32, CQ)))

                ar_handles = []
                s_ps = psB.tile([32, CQ], F32, tag="sacc")
                for s in range(NSWEEP):
                    if s == SPLIT_SWEEP:
                        ar_handles.append(flush_and_allreduce(s_ps, f"r{r}a"))
                        s_ps = psB.tile([32, CQ], F32, tag="sacc")
                    first, last_s = (s == 0 or s == SPLIT_SWEEP), \
                        (s == SPLIT_SWEEP - 1 or s == NSWEEP - 1)
                    if last:
                        it = ihw_pool.tile([128, CQ], F32, tag="it_w")
                        nc.sync.dma_start(it[:], ih_d[2 * s:2 * (s + 1)])
                    else:
                        it = ih_pool.tile([128, CQ], BF16, tag="it_l")
                        nc.sync.dma_start(it[:], ihb_d[2 * s:2 * (s + 1)])
                    pr = prod_pool.tile(
                        [128, CQ], SDT, tag="pr_f" if last else "pr_b")
                    nc.vector.tensor_mul(pr[:], it[:], v4[:])
                    dlog = small_pool.tile([128, C], F32, tag="dlog")
                    nc.vector.reduce_sum(
                        dlog[:], pr[:].rearrange("p (c q) -> p c q", q=Q),
                        axis=AX.X)
                    bsl = b_acc[:, 32 * s:32 * (s + 1)]
                    nc.vector.tensor_add(bsl, bsl, dlog[:])
                    # softmax over c: coef = exp(b) / Z; 1/Z is folded into
                    # the selector matmul weights, exp broadcast to (c,q).
                    e = small_pool.tile([128, C], F32, tag="e")
                    nc.scalar.activation(e[:], bsl, AF.Exp)
                    z = small_pool.tile([128, 1], F32, tag="z")
                    nc.vector.reduce_sum(z[:], e[:], axis=AX.X)
                    rz = small_pool.tile([128, 1], F32, tag="rz")
                    nc.vector.reciprocal(rz[:], z[:])
                    selz = small_pool.tile([128, 32], SDT, tag="selz")
                    nc.vector.tensor_scalar_mul(
                        selz[:], sel_t[:] if last else sel_bf[:], rz[:])
                    e4 = wt_pool.tile([128, CQ], SDT, tag="e4")
                    nc.scalar.activation(
                        e4[:].rearrange("p (c q) -> p c q", q=Q),
                        e[:].unsqueeze(-1).broadcast_to((128, C, Q)), AF.Copy)
                    wtt = wt_pool.tile([128, CQ], SDT, tag="wtt")
                    nc.vector.tensor_mul(wtt[:], it[:], e4[:])
                    for h in range(4):
                        sl = slice(512 * h, 512 * (h + 1))
                        nc.tensor.matmul(
                            s_ps[:, sl], selz[:], wtt[:, sl],
                            start=first, stop=last_s)
                ar_handles.append(flush_and_allreduce(s_ps, f"r{r}b"))

    nc.compile()
    return nc


def _split_bf16(a):
    hi = a.astype(BF)
    lo = (a - hi.astype(np.float32)).astype(BF)
    return hi, lo


def _prep_core_inputs(x, W):
    """Host-side shard + repack for one call. Returns list of in_maps."""
    xs_all = np.ascontiguousarray(x)          # [B, I, P]
    in_maps = []
    sel = np.tile(np.eye(32, dtype=np.float32), (4, 1))
    for k in range(N_CORES):
        xs = xs_all[:, k * IL:(k + 1) * IL, :]          # [B, IL, P]
        # lhsT[t][i2*64+p, col]:
        #   cols 0:32   <- xs[b, 2t, p]        (block i2=0)
        #   cols 32:64  <- xs[b, 2t+1, p]      (block i2=1)
        #   cols 64:96  <- xs[b, 2t+i2, p]/32  (both blocks: s0 accumulation)
        xt = xs.transpose(1, 2, 0).reshape(NPAIR, 2, P, B)  # [t, i2, p, b]
        lhsT = np.zeros((NPAIR, 128, 96), np.float32)
        lhsT[:, 0:64, 0:32] = xt[:, 0]
        lhsT[:, 64:128, 32:64] = xt[:, 1]
        lhsT[:, 0:64, 64:96] = xt[:, 0] / C
        lhsT[:, 64:128, 64:96] = xt[:, 1] / C
        lhi, llo = _split_bf16(lhsT)
        if CONFIG["p1_single"]:
            lall = lhi                                   # [t, 128, 96]
        else:
            lall = np.concatenate([lhi, llo], axis=2)    # [t, 128, 192]
        lall = np.ascontiguousarray(lall.transpose(1, 0, 2)).reshape(128, -1)
        Ws = W[k * IL:(k + 1) * IL]                      # [IL, C, P, Q]
        w_rhs = np.ascontiguousarray(
            Ws.reshape(NPAIR, 2, C, P, Q).transpose(0, 1, 3, 2, 4)
        ).reshape(NPAIR, 128, CQ)
        whi, wlo = _split_bf16(w_rhs)
        if CONFIG["p1_single"]:
            w2 = np.ascontiguousarray(whi)               # [t, 128, 2048]
        else:
            w2 = np.concatenate([whi, wlo], axis=2)      # [t, 128, 4096]
        in_maps.append({"lhsT": lall, "w_rhs": w2, "sel": sel})
    return in_maps


def _host_reference(x, W, bias):
    """Exact numpy fallback (used only if bias != 0, which the problem's
    input spec says cannot happen; the device kernel assumes uniform
    round-0 routing coefficients)."""
    ih = np.einsum("bip,icpq->bicq", x, W)
    b = bias.astype(np.float64)
    out = None
    for r in range(NUM_ROUTING):
        e = np.exp(b - b.max(axis=2, keepdims=True))
        c = e / e.sum(axis=2, keepdims=True)
        s = (c * ih).sum(axis=1, keepdims=True)
        sq = np.sum(s * s, axis=-1, keepdims=True)
        out = s * (sq / (1.0 + sq) / np.sqrt(sq))
        if r != NUM_ROUTING - 1:
            b = b + np.sum(ih * out, axis=-1, keepdims=True)
    return out.reshape(B, C, Q).astype(np.float32)


def kernel(x, W, bias):
    global _compiled, _compiled_cfg
    x = np.asarray(x, dtype=np.float32)
    W = np.asarray(W, dtype=np.float32)
    bias = np.asarray(bias, dtype=np.float32)
    if np.any(bias):
        return _host_reference(x, W, bias)

    cfg = (CONFIG["r2_f32"], CONFIG["p1_single"])
    if _compiled is None or _compiled_cfg != cfg:
        _compiled = _build_kernel()
        _compiled_cfg = cfg
    nc = _compiled

    in_maps = _prep_core_inputs(x, W)
    res = run_bass_kernel_spmd(
        nc, in_maps, list(range(N_CORES)),
        trace=CONFIG["trace"], trace_cores=CONFIG["trace_cores"],
    )
    kernel.last_results = res
    out = res.results[0]["out"].reshape(B, C, Q)
    return out


# revision 33
# speedup vs baseline: 1.0081x; 1.0081x over previous
"""Trainium2 Bass kernel for CapsuleLayer (dynamic routing) on 8 NeuronCores.

Problem: x[32,1152,64], W[1152,32,64,64], bias[1,1152,32,1] (zeros) ->
         out[32,32,64]
  inputs_hat = einsum('bip,icpq->bicq', x, W)
  3 rounds of routing (softmax over capsule axis, squash, agreement update).

Sharding: input-capsule axis i=1152 split over 8 cores (144 each).
  - W read exactly once per core (75.5 MB), ih materialized to HBM per core.
  - softmax over output capsules c is core-local; only the [32,32,64]
    pre-squash sum is AllReduced per routing round (split into two
    AllReduces so the first overlaps the tail of the round's compute).

Phase 1 per core: 72 block-diagonal matmul pairs (2 i's packed -> K=128)
  compute ih[b,i,c,q]. x and W are split hi/lo into bf16 on the host;
  ih = x_hi@W_hi + x_hi@W_lo + x_lo@W_hi runs at full PE rate with ~1e-5
  relative error. Extra lhsT columns accumulate s0 = sum_i ih / 32 in PSUM
  (bias==0 so round-0 routing coefficients are uniform 1/32). ih is stored
  to HBM in bf16 (round-1 logit pass) and fp32 (final round).

Routing rounds r=1,2: one fused pass per 4-i sweep over ih:
  logits += <ih, v> (DVE mult + reduce over q), softmax over c via exp on
  ScalarE broadcast to the (c,q) grid, s += exp * ih with the 1/Z softmax
  denominator folded into the selector matmul weights (PSUM accumulation).
  Round 1 runs in bf16 (it only steers coefficients); the final round runs
  fp32 end-to-end from the fp32 ih copy.
"""

import os
import sys

import numpy as np

for _p in (
    "/opt/trn_rl_repo",
    "/root/.axon_site",
    "/root/.axon_site/_ro/trn_rl_repo",
    "/root/.axon_site/_ro/pypackages",
):
    if os.path.isdir(_p) and _p not in sys.path:
        sys.path.append(_p)

import ml_dtypes
import concourse.bacc as bacc
import concourse.mybir as mybir
import concourse.tile as tile
from concourse.bass_utils import run_bass_kernel_spmd

F32 = mybir.dt.float32
BF16 = mybir.dt.bfloat16
AF = mybir.ActivationFunctionType
AX = mybir.AxisListType
ALU = mybir.AluOpType
BF = ml_dtypes.bfloat16

B, I, P, C, Q = 32, 1152, 64, 32, 64
N_CORES = 8
IL = I // N_CORES          # 144 input capsules per core
NPAIR = IL // 2            # 72 matmul pairs
NSWEEP = IL // 4           # 36 routing sweeps (4 i's x 32 b = 128 partitions)
CQ = C * Q                 # 2048
NUM_ROUTING = 3
SPLIT_PAIR = 56            # s0 partial AllReduced after this many pairs
SPLIT_SWEEP = 30           # routing partial AllReduced after this many sweeps

CONFIG = {
    "r2_f32": False,          # final round weighted sum in fp32 (else bf16)
    "p1_single": True,        # phase-1 matmul = x_hi @ W_hi only (skip the
                              # hi/lo correction terms: ih is consumed through
                              # its bf16 copy anyway)
    "trace": False,           # profile the run (exec_time_ns); needs ntff hook
    "trace_cores": None,      # None -> core 0 only
}

_compiled = None
_compiled_cfg = None


def _build_kernel():
    """Build + compile the SPMD Bass module (identical program on 8 cores)."""
    nc = bacc.Bacc("TRN2", target_bir_lowering=False, debug=False,
                   num_devices=N_CORES)

    p1s = CONFIG["p1_single"]
    lstride = 96 if p1s else 192
    # lhsT_all[p, t*stride + (0:96)] = hi half, (96:192) = lo half for pair t
    lall_d = nc.dram_tensor("lhsT", [128, NPAIR * lstride], BF16,
                            kind="ExternalInput")
    # w2[t, p, 0:2048] = hi, [t, p, 2048:4096] = lo (lo absent if p1_single)
    w_d = nc.dram_tensor("w_rhs", [NPAIR, 128, CQ if p1s else 2 * CQ], BF16,
                         kind="ExternalInput")
    sel_d = nc.dram_tensor("sel", [128, 32], F32, kind="ExternalInput")
    out_d = nc.dram_tensor("out", [B, CQ], F32, kind="ExternalOutput")

    r2f = CONFIG["r2_f32"]
    assert not (r2f and CONFIG["p1_single"]), \
        "r2_f32 needs the fp32 ih copy from the multi-term phase 1"
    ih_d = nc.dram_tensor("ih_buf", [NPAIR, 64, CQ], F32) if r2f else None
    ihb_d = nc.dram_tensor("ihb_buf", [NPAIR, 64, CQ], BF16)

    rgroups = [list(range(N_CORES))]

    with tile.TileContext(nc) as tc:
        with (
            tc.tile_pool(name="lall", bufs=1) as lall_pool,
            tc.tile_pool(name="w", bufs=3) as w_pool,
            tc.tile_pool(name="cp", bufs=2) as cp_pool,
            tc.tile_pool(name="ih", bufs=4) as ih_pool,
            tc.tile_pool(name="ihw", bufs=2) as ihw_pool,
            tc.tile_pool(name="prod", bufs=3) as prod_pool,
            tc.tile_pool(name="wt", bufs=3) as wt_pool,
            tc.tile_pool(name="v4", bufs=1) as v4_pool,
            tc.tile_pool(name="small", bufs=3) as small_pool,
            tc.tile_pool(name="acc", bufs=1) as acc_pool,
            tc.tile_pool(name="sv", bufs=1) as sv_pool,
            tc.tile_pool(name="psA", bufs=4, space="PSUM") as psA,
            tc.tile_pool(name="psB", bufs=1, space="PSUM") as psB,
            tc.tile_pool(name="dram", bufs=2, space="DRAM") as dram_pool,
        ):
            sel_t = small_pool.tile([128, 32], F32, tag="sel")
            nc.sync.dma_start(sel_t[:], sel_d[:])
            sel_bf = small_pool.tile([128, 32], BF16, tag="selbf")
            nc.vector.tensor_copy(sel_bf[:], sel_t[:])

            b_acc = acc_pool.tile([128, NSWEEP * 32], F32, tag="bacc")
            nc.vector.memset(b_acc[:], 0.0)

            lall = lall_pool.tile([128, NPAIR * lstride], BF16, tag="lall")
            nc.sync.dma_start(lall[:], lall_d[:])

            def flush_and_allreduce(s_ps, tag):
                """PSUM partial -> SBUF -> DRAM -> AllReduce. Returns the
                collective's DRAM output tile."""
                f_sb = sv_pool.tile([32, CQ], F32, tag="s_sb")
                nc.scalar.copy(f_sb[:], s_ps[:])
                a_in = dram_pool.tile([32, CQ], F32, tag="ar_in")
                a_out = dram_pool.tile([32, CQ], F32, tag="ar_out")
                nc.gpsimd.dma_start(a_in[:], f_sb[:])
                nc.gpsimd.collective_compute(
                    "AllReduce", ALU.add,
                    ins=[a_in[:].opt()], outs=[a_out[:].opt()],
                    replica_groups=rgroups,
                )
                return a_out

            # ---------------- Phase 1: ih = x @ W, s0 = sum_i ih / 32 -------
            s_ps = psB.tile([32, CQ], F32, tag="sacc")
            ar_handles = []
            for t in range(NPAIR):
                if t == SPLIT_PAIR:
                    ar_handles.append(flush_and_allreduce(s_ps, "s0a"))
                    s_ps = psB.tile([32, CQ], F32, tag="sacc")
                first, last_t = (t == 0 or t == SPLIT_PAIR), \
                    (t == SPLIT_PAIR - 1 or t == NPAIR - 1)
                lhi = lall[:, t * lstride:t * lstride + 96]
                wt_ = w_pool.tile([128, CQ if p1s else 2 * CQ], BF16)
                nc.sync.dma_start(wt_[:], w_d[t])
                cp = None
                if not p1s:
                    llo = lall[:, t * lstride + 96:t * lstride + 192]
                    cp = cp_pool.tile([64, CQ], F32, tag="cp32")
                cpb = cp_pool.tile([64, CQ], BF16, tag="cpb")
                for blk in range(4):
                    sl = slice(512 * blk, 512 * (blk + 1))
                    ps = psA.tile([64, 512], F32)
                    whi = wt_[:, sl]
                    if p1s:
                        nc.tensor.matmul(ps[:], lhi[:, 0:64], whi,
                                         start=True, stop=True)
                    else:
                        wlo = wt_[:, CQ + 512 * blk:CQ + 512 * (blk + 1)]
                        nc.tensor.matmul(ps[:], lhi[:, 0:64], whi,
                                         start=True, stop=False)
                        nc.tensor.matmul(ps[:], lhi[:, 0:64], wlo,
                                         start=False, stop=False)
                        nc.tensor.matmul(ps[:], llo[:, 0:64], whi,
                                         start=False, stop=True)
                    nc.tensor.matmul(
                        s_ps[:, sl], lhi[:, 64:96], whi,
                        start=first, stop=last_t,
                    )
                    if p1s:
                        # alternate ACT/DVE so neither becomes the pipeline
                        # stage bound (DVE is otherwise idle in phase 1)
                        if blk % 2 == 0:
                            nc.scalar.copy(cpb[:, sl], ps[:])
                        else:
                            nc.vector.tensor_copy(cpb[:, sl], ps[:])
                    else:
                        nc.scalar.copy(cp[:, sl], ps[:])
                if not p1s:
                    nc.vector.tensor_copy(cpb[:], cp[:])
                    if r2f:
                        nc.sync.dma_start(ih_d[t], cp[:])
                nc.sync.dma_start(ihb_d[t], cpb[:])
            ar_handles.append(flush_and_allreduce(s_ps, "s0b"))

            # ---------------- Routing rounds -------------------------------
            # The [32, b*(c,q)] sum is reshaped to [128, 512] (partition =
            # (b, cq-quarter)) so the squash math runs on all 128 lanes.
            CQ4 = CQ // 4   # 512
            C4 = C // 4     # 8 capsules per partition-quarter
            for r in range(1, NUM_ROUTING + 1):
                # combine the two AllReduce halves, then squash -> v
                pa = sv_pool.tile([128, CQ4], F32, tag="pa")
                pb = sv_pool.tile([128, CQ4], F32, tag="pb")
                nc.gpsimd.dma_start(pa[:], ar_handles[0][:])
                nc.gpsimd.dma_start(pb[:], ar_handles[1][:])
                S_sb = sv_pool.tile([128, CQ4], F32, tag="S_sb")
                nc.vector.tensor_add(S_sb[:], pa[:], pb[:])

                # squash: v = S * sqrt(sq)/(1+sq),  sq = sum_q S^2
                S3 = S_sb[:].rearrange("b (c q) -> b c q", q=Q)
                sq = small_pool.tile([128, C4], F32, tag="sq")
                sqr = sv_pool.tile([128, CQ4], F32, tag="sqr")
                nc.vector.tensor_mul(sqr[:], S_sb[:], S_sb[:])
                nc.vector.reduce_sum(
                    sq[:], sqr[:].rearrange("b (c q) -> b c q", q=Q),
                    axis=AX.X)
                rt = small_pool.tile([128, C4], F32, tag="rt")
                nc.scalar.sqrt(rt[:], sq[:])
                onep = small_pool.tile([128, C4], F32, tag="onep")
                nc.vector.tensor_scalar_add(onep[:], sq[:], 1.0)
                rden = small_pool.tile([128, C4], F32, tag="rden")
                nc.vector.reciprocal(rden[:], onep[:])
                scale = small_pool.tile([128, C4], F32, tag="scale")
                nc.vector.tensor_mul(scale[:], rt[:], rden[:])
                # v overwrites S_sb in place
                nc.vector.tensor_mul(
                    S3, S3, scale[:].unsqueeze(-1).broadcast_to((128, C4, Q)))
                v_t = S_sb

                if r == NUM_ROUTING:
                    nc.sync.dma_start(out_d[:], v_t[:])
                    break

                last = (r == NUM_ROUTING - 1) and r2f
                SDT = F32 if last else BF16   # sweep compute dtype

                # broadcast v across the 4 i-slots of the 128 partitions
                # (DRAM bounce; the stride-0 broadcast read is legal there)
                v_c = sv_pool.tile([128, CQ4], SDT, tag="v_c")
                nc.vector.tensor_copy(v_c[:], v_t[:])
                vb = dram_pool.tile([32, CQ], SDT, tag="vb")
                nc.gpsimd.dma_start(vb[:], v_c[:])
                v4 = v4_pool.tile([128, CQ], SDT, tag="v4")
                nc.gpsimd.dma_start(
                    v4[:], vb[:].unsqueeze(0).broadcast_to((4, 32, CQ)))

                ar_handles = []
                s_ps = psB.tile([32, CQ], F32, tag="sacc")
                for s in range(NSWEEP):
                    if s == SPLIT_SWEEP:
                        ar_handles.append(flush_and_allreduce(s_ps, f"r{r}a"))
                        s_ps = psB.tile([32, CQ], F32, tag="sacc")
                    first, last_s = (s == 0 or s == SPLIT_SWEEP), \
                        (s == SPLIT_SWEEP - 1 or s == NSWEEP - 1)
                    if last:
                        it = ihw_pool.tile([128, CQ], F32, tag="it_w")
                        nc.sync.dma_start(it[:], ih_d[2 * s:2 * (s + 1)])
                    else:
                        it = ih_pool.tile([128, CQ], BF16, tag="it_l")
                        nc.sync.dma_start(it[:], ihb_d[2 * s:2 * (s + 1)])
                    pr = prod_pool.tile(
                        [128, CQ], SDT, tag="pr_f" if last else "pr_b")
                    nc.vector.tensor_mul(pr[:], it[:], v4[:])
                    dlog = small_pool.tile([128, C], F32, tag="dlog")
                    nc.vector.reduce_sum(
                        dlog[:], pr[:].rearrange("p (c q) -> p c q", q=Q),
                        axis=AX.X)
                    bsl = b_acc[:, 32 * s:32 * (s + 1)]
                    nc.vector.tensor_add(bsl, bsl, dlog[:])
                    # softmax over c: coef = exp(b) / Z; 1/Z is folded into
                    # the selector matmul weights, exp broadcast to (c,q).
                    e = small_pool.tile([128, C], F32, tag="e")
                    nc.scalar.activation(e[:], bsl, AF.Exp)
                    z = small_pool.tile([128, 1], F32, tag="z")
                    nc.vector.reduce_sum(z[:], e[:], axis=AX.X)
                    rz = small_pool.tile([128, 1], F32, tag="rz")
                    nc.vector.reciprocal(rz[:], z[:])
                    selz = small_pool.tile([128, 32], SDT, tag="selz")
                    nc.vector.tensor_scalar_mul(
                        selz[:], sel_t[:] if last else sel_bf[:], rz[:])
                    e4 = wt_pool.tile([128, CQ], SDT, tag="e4")
                    nc.scalar.activation(
                        e4[:].rearrange("p (c q) -> p c q", q=Q),
                        e[:].unsqueeze(-1).broadcast_to((128, C, Q)), AF.Copy)
                    wtt = wt_pool.tile([128, CQ], SDT, tag="wtt")
                    nc.vector.tensor_mul(wtt[:], it[:], e4[:])
                    for h in range(4):
                        sl = slice(512 * h, 512 * (h + 1))
                        nc.tensor.matmul(
                            s_ps[:, sl], selz[:], wtt[:, sl],
                            start=first, stop=last_s)
                ar_handles.append(flush_and_allreduce(s_ps, f"r{r}b"))

    nc.compile()
    return nc


def _split_bf16(a):
    hi = a.astype(BF)
    lo = (a - hi.astype(np.float32)).astype(BF)
    return hi, lo


def _prep_core_inputs(x, W):
    """Host-side shard + repack for one call. Returns list of in_maps."""
    xs_all = np.ascontiguousarray(x)          # [B, I, P]
    in_maps = []
    sel = np.tile(np.eye(32, dtype=np.float32), (4, 1))
    for k in range(N_CORES):
        xs = xs_all[:, k * IL:(k + 1) * IL, :]          # [B, IL, P]
        # lhsT[t][i2*64+p, col]:
        #   cols 0:32   <- xs[b, 2t, p]        (block i2=0)
        #   cols 32:64  <- xs[b, 2t+1, p]      (block i2=1)
        #   cols 64:96  <- xs[b, 2t+i2, p]/32  (both blocks: s0 accumulation)
        xt = xs.transpose(1, 2, 0).reshape(NPAIR, 2, P, B)  # [t, i2, p, b]
        lhsT = np.zeros((NPAIR, 128, 96), np.float32)
        lhsT[:, 0:64, 0:32] = xt[:, 0]
        lhsT[:, 64:128, 32:64] = xt[:, 1]
        lhsT[:, 0:64, 64:96] = xt[:, 0] / C
        lhsT[:, 64:128, 64:96] = xt[:, 1] / C
        lhi, llo = _split_bf16(lhsT)
        if CONFIG["p1_single"]:
            lall = lhi                                   # [t, 128, 96]
        else:
            lall = np.concatenate([lhi, llo], axis=2)    # [t, 128, 192]
        lall = np.ascontiguousarray(lall.transpose(1, 0, 2)).reshape(128, -1)
        Ws = W[k * IL:(k + 1) * IL]                      # [IL, C, P, Q]
        w_rhs = np.ascontiguousarray(
            Ws.reshape(NPAIR, 2, C, P, Q).transpose(0, 1, 3, 2, 4)
        ).reshape(NPAIR, 128, CQ)
        whi, wlo = _split_bf16(w_rhs)
        if CONFIG["p1_single"]:
            w2 = np.ascontiguousarray(whi)               # [t, 128, 2048]
        else:
            w2 = np.concatenate([whi, wlo], axis=2)      # [t, 128, 4096]
        in_maps.append({"lhsT": lall, "w_rhs": w2, "sel": sel})
    return in_maps


def _host_reference(x, W, bias):
    """Exact numpy fallback (used only if bias != 0, which the problem's
    input spec says cannot happen; the device kernel assumes uniform
    round-0 routing coefficients)."""
    ih = np.einsum("bip,icpq->bicq", x, W)
    b = bias.astype(np.float64)
    out = None
    for r in range(NUM_ROUTING):
        e = np.exp(b - b.max(axis=2, keepdims=True))
        c = e / e.sum(axis=2, keepdims=True)
        s = (c * ih).sum(axis=1, keepdims=True)
        sq = np.sum(s * s, axis=-1, keepdims=True)
        out = s * (sq / (1.0 + sq) / np.sqrt(sq))
        if r != NUM_ROUTING - 1:
            b = b + np.sum(ih * out, axis=-1, keepdims=True)
    return out.reshape(B, C, Q).astype(np.float32)


def kernel(x, W, bias):
    global _compiled, _compiled_cfg
    x = np.asarray(x, dtype=np.float32)
    W = np.asarray(W, dtype=np.float32)
    bias = np.asarray(bias, dtype=np.float32)
    if np.any(bias):
        return _host_reference(x, W, bias)

    cfg = (CONFIG["r2_f32"], CONFIG["p1_single"])
    if _compiled is None or _compiled_cfg != cfg:
        _compiled = _build_kernel()
        _compiled_cfg = cfg
    nc = _compiled

    in_maps = _prep_core_inputs(x, W)
    res = run_bass_kernel_spmd(
        nc, in_maps, list(range(N_CORES)),
        trace=CONFIG["trace"], trace_cores=CONFIG["trace_cores"],
    )
    kernel.last_results = res
    out = res.results[0]["out"].reshape(B, C, Q)
    return out


# revision 35
# speedup vs baseline: 1.0086x; 1.0005x over previous
"""Trainium2 Bass kernel for CapsuleLayer (dynamic routing) on 8 NeuronCores.

Problem: x[32,1152,64], W[1152,32,64,64], bias[1,1152,32,1] (zeros) ->
         out[32,32,64]
  inputs_hat = einsum('bip,icpq->bicq', x, W)
  3 rounds of routing (softmax over capsule axis, squash, agreement update).

Sharding: input-capsule axis i=1152 split over 8 cores (144 each).
  - W read exactly once per core (75.5 MB), ih materialized to HBM per core.
  - softmax over output capsules c is core-local; only the [32,32,64]
    pre-squash sum is AllReduced per routing round (split into two
    AllReduces so the first overlaps the tail of the round's compute).

Phase 1 per core: 72 block-diagonal matmul pairs (2 i's packed -> K=128)
  compute ih[b,i,c,q]. x and W are split hi/lo into bf16 on the host;
  ih = x_hi@W_hi + x_hi@W_lo + x_lo@W_hi runs at full PE rate with ~1e-5
  relative error. Extra lhsT columns accumulate s0 = sum_i ih / 32 in PSUM
  (bias==0 so round-0 routing coefficients are uniform 1/32). ih is stored
  to HBM in bf16 (round-1 logit pass) and fp32 (final round).

Routing rounds r=1,2: one fused pass per 4-i sweep over ih:
  logits += <ih, v> (DVE mult + reduce over q), softmax over c via exp on
  ScalarE broadcast to the (c,q) grid, s += exp * ih with the 1/Z softmax
  denominator folded into the selector matmul weights (PSUM accumulation).
  Round 1 runs in bf16 (it only steers coefficients); the final round runs
  fp32 end-to-end from the fp32 ih copy.
"""

import os
import sys

import numpy as np

for _p in (
    "/opt/trn_rl_repo",
    "/root/.axon_site",
    "/root/.axon_site/_ro/trn_rl_repo",
    "/root/.axon_site/_ro/pypackages",
):
    if os.path.isdir(_p) and _p not in sys.path:
        sys.path.append(_p)

import ml_dtypes
import concourse.bacc as bacc
import concourse.mybir as mybir
import concourse.tile as tile
from concourse.bass_utils import run_bass_kernel_spmd

F32 = mybir.dt.float32
BF16 = mybir.dt.bfloat16
AF = mybir.ActivationFunctionType
AX = mybir.AxisListType
ALU = mybir.AluOpType
BF = ml_dtypes.bfloat16

B, I, P, C, Q = 32, 1152, 64, 32, 64
N_CORES = 8
IL = I // N_CORES          # 144 input capsules per core
NPAIR = IL // 2            # 72 matmul pairs
NSWEEP = IL // 4           # 36 routing sweeps (4 i's x 32 b = 128 partitions)
CQ = C * Q                 # 2048
NUM_ROUTING = 3
SPLIT_PAIR = 56            # s0 partial AllReduced after this many pairs
SPLIT_SWEEP = 30           # routing partial AllReduced after this many sweeps

CONFIG = {
    "r2_f32": False,          # final round weighted sum in fp32 (else bf16)
    "p1_single": True,        # phase-1 matmul = x_hi @ W_hi only (skip the
                              # hi/lo correction terms: ih is consumed through
                              # its bf16 copy anyway)
    "trace": False,           # profile the run (exec_time_ns); needs ntff hook
    "trace_cores": None,      # None -> core 0 only
}

_compiled = None
_compiled_cfg = None


def _build_kernel():
    """Build + compile the SPMD Bass module (identical program on 8 cores)."""
    nc = bacc.Bacc("TRN2", target_bir_lowering=False, debug=False,
                   num_devices=N_CORES)

    p1s = CONFIG["p1_single"]
    lstride = 96 if p1s else 192
    # lhsT_all[p, t*stride + (0:96)] = hi half, (96:192) = lo half for pair t
    lall_d = nc.dram_tensor("lhsT", [128, NPAIR * lstride], BF16,
                            kind="ExternalInput")
    # w2[t, p, 0:2048] = hi, [t, p, 2048:4096] = lo (lo absent if p1_single)
    w_d = nc.dram_tensor("w_rhs", [NPAIR, 128, CQ if p1s else 2 * CQ], BF16,
                         kind="ExternalInput")
    sel_d = nc.dram_tensor("sel", [128, 32], F32, kind="ExternalInput")
    out_d = nc.dram_tensor("out", [B, CQ], F32, kind="ExternalOutput")

    r2f = CONFIG["r2_f32"]
    assert not (r2f and CONFIG["p1_single"]), \
        "r2_f32 needs the fp32 ih copy from the multi-term phase 1"
    ih_d = nc.dram_tensor("ih_buf", [NPAIR, 64, CQ], F32) if r2f else None
    ihb_d = nc.dram_tensor("ihb_buf", [NPAIR, 64, CQ], BF16)

    rgroups = [list(range(N_CORES))]

    with tile.TileContext(nc) as tc:
        with (
            tc.tile_pool(name="lall", bufs=1) as lall_pool,
            tc.tile_pool(name="w", bufs=5) as w_pool,
            tc.tile_pool(name="cp", bufs=3) as cp_pool,
            tc.tile_pool(name="ih", bufs=8) as ih_pool,
            tc.tile_pool(name="ihw", bufs=2) as ihw_pool,
            tc.tile_pool(name="prod", bufs=4) as prod_pool,
            tc.tile_pool(name="wt", bufs=4) as wt_pool,
            tc.tile_pool(name="v4", bufs=1) as v4_pool,
            tc.tile_pool(name="small", bufs=3) as small_pool,
            tc.tile_pool(name="acc", bufs=1) as acc_pool,
            tc.tile_pool(name="sv", bufs=1) as sv_pool,
            tc.tile_pool(name="psA", bufs=4, space="PSUM") as psA,
            tc.tile_pool(name="psB", bufs=1, space="PSUM") as psB,
            tc.tile_pool(name="dram", bufs=2, space="DRAM") as dram_pool,
        ):
            sel_t = small_pool.tile([128, 32], F32, tag="sel")
            nc.sync.dma_start(sel_t[:], sel_d[:])
            sel_bf = small_pool.tile([128, 32], BF16, tag="selbf")
            nc.vector.tensor_copy(sel_bf[:], sel_t[:])

            b_acc = acc_pool.tile([128, NSWEEP * 32], F32, tag="bacc")
            nc.vector.memset(b_acc[:], 0.0)

            lall = lall_pool.tile([128, NPAIR * lstride], BF16, tag="lall")
            nc.sync.dma_start(lall[:], lall_d[:])

            def flush_and_allreduce(s_ps, tag):
                """PSUM partial -> SBUF -> DRAM -> AllReduce. Returns the
                collective's DRAM output tile."""
                f_sb = sv_pool.tile([32, CQ], F32, tag="s_sb")
                nc.scalar.copy(f_sb[:], s_ps[:])
                a_in = dram_pool.tile([32, CQ], F32, tag="ar_in")
                a_out = dram_pool.tile([32, CQ], F32, tag="ar_out")
                nc.gpsimd.dma_start(a_in[:], f_sb[:])
                nc.gpsimd.collective_compute(
                    "AllReduce", ALU.add,
                    ins=[a_in[:].opt()], outs=[a_out[:].opt()],
                    replica_groups=rgroups,
                )
                return a_out

            # ---------------- Phase 1: ih = x @ W, s0 = sum_i ih / 32 -------
            s_ps = psB.tile([32, CQ], F32, tag="sacc")
            ar_handles = []
            for t in range(NPAIR):
                if t == SPLIT_PAIR:
                    ar_handles.append(flush_and_allreduce(s_ps, "s0a"))
                    s_ps = psB.tile([32, CQ], F32, tag="sacc")
                first, last_t = (t == 0 or t == SPLIT_PAIR), \
                    (t == SPLIT_PAIR - 1 or t == NPAIR - 1)
                lhi = lall[:, t * lstride:t * lstride + 96]
                wt_ = w_pool.tile([128, CQ if p1s else 2 * CQ], BF16)
                nc.sync.dma_start(wt_[:], w_d[t])
                cp = None
                if not p1s:
                    llo = lall[:, t * lstride + 96:t * lstride + 192]
                    cp = cp_pool.tile([64, CQ], F32, tag="cp32")
                cpb = cp_pool.tile([64, CQ], BF16, tag="cpb")
                if p1s:
                    # group same-lhsT matmuls so LDWEIGHTS doesn't alternate
                    pss = []
                    for blk in range(4):
                        sl = slice(512 * blk, 512 * (blk + 1))
                        ps = psA.tile([64, 512], F32)
                        pss.append(ps)
                        nc.tensor.matmul(ps[:], lhi[:, 0:64], wt_[:, sl],
                                         start=True, stop=True)
                    for blk in range(4):
                        sl = slice(512 * blk, 512 * (blk + 1))
                        nc.tensor.matmul(
                            s_ps[:, sl], lhi[:, 64:96], wt_[:, sl],
                            start=first, stop=last_t,
                        )
                        # alternate ACT/DVE so neither becomes the pipeline
                        # stage bound (DVE is otherwise idle in phase 1)
                        if blk % 2 == 0:
                            nc.scalar.copy(cpb[:, sl], pss[blk][:])
                        else:
                            nc.vector.tensor_copy(cpb[:, sl], pss[blk][:])
                else:
                    for blk in range(4):
                        sl = slice(512 * blk, 512 * (blk + 1))
                        ps = psA.tile([64, 512], F32)
                        whi = wt_[:, sl]
                        wlo = wt_[:, CQ + 512 * blk:CQ + 512 * (blk + 1)]
                        nc.tensor.matmul(ps[:], lhi[:, 0:64], whi,
                                         start=True, stop=False)
                        nc.tensor.matmul(ps[:], lhi[:, 0:64], wlo,
                                         start=False, stop=False)
                        nc.tensor.matmul(ps[:], llo[:, 0:64], whi,
                                         start=False, stop=True)
                        nc.tensor.matmul(
                            s_ps[:, sl], lhi[:, 64:96], whi,
                            start=first, stop=last_t,
                        )
                        nc.scalar.copy(cp[:, sl], ps[:])
                if not p1s:
                    nc.vector.tensor_copy(cpb[:], cp[:])
                    if r2f:
                        nc.sync.dma_start(ih_d[t], cp[:])
                nc.sync.dma_start(ihb_d[t], cpb[:])
            ar_handles.append(flush_and_allreduce(s_ps, "s0b"))

            # ---------------- Routing rounds -------------------------------
            # The [32, b*(c,q)] sum is reshaped to [128, 512] (partition =
            # (b, cq-quarter)) so the squash math runs on all 128 lanes.
            CQ4 = CQ // 4   # 512
            C4 = C // 4     # 8 capsules per partition-quarter
            for r in range(1, NUM_ROUTING + 1):
                # combine the two AllReduce halves, then squash -> v
                pa = sv_pool.tile([128, CQ4], F32, tag="pa")
                pb = sv_pool.tile([128, CQ4], F32, tag="pb")
                nc.gpsimd.dma_start(pa[:], ar_handles[0][:])
                nc.gpsimd.dma_start(pb[:], ar_handles[1][:])
                S_sb = sv_pool.tile([128, CQ4], F32, tag="S_sb")
                nc.vector.tensor_add(S_sb[:], pa[:], pb[:])

                # squash: v = S * sqrt(sq)/(1+sq),  sq = sum_q S^2
                S3 = S_sb[:].rearrange("b (c q) -> b c q", q=Q)
                sq = small_pool.tile([128, C4], F32, tag="sq")
                sqr = sv_pool.tile([128, CQ4], F32, tag="sqr")
                nc.vector.tensor_mul(sqr[:], S_sb[:], S_sb[:])
                nc.vector.reduce_sum(
                    sq[:], sqr[:].rearrange("b (c q) -> b c q", q=Q),
                    axis=AX.X)
                rt = small_pool.tile([128, C4], F32, tag="rt")
                nc.scalar.sqrt(rt[:], sq[:])
                onep = small_pool.tile([128, C4], F32, tag="onep")
                nc.vector.tensor_scalar_add(onep[:], sq[:], 1.0)
                rden = small_pool.tile([128, C4], F32, tag="rden")
                nc.vector.reciprocal(rden[:], onep[:])
                scale = small_pool.tile([128, C4], F32, tag="scale")
                nc.vector.tensor_mul(scale[:], rt[:], rden[:])
                # v overwrites S_sb in place
                nc.vector.tensor_mul(
                    S3, S3, scale[:].unsqueeze(-1).broadcast_to((128, C4, Q)))
                v_t = S_sb

                if r == NUM_ROUTING:
                    nc.sync.dma_start(out_d[:], v_t[:])
                    break

                last = (r == NUM_ROUTING - 1) and r2f
                SDT = F32 if last else BF16   # sweep compute dtype

                # broadcast v across the 4 i-slots of the 128 partitions
                # (DRAM bounce; the stride-0 broadcast read is legal there)
                v_c = sv_pool.tile([128, CQ4], SDT, tag="v_c")
                nc.vector.tensor_copy(v_c[:], v_t[:])
                vb = dram_pool.tile([32, CQ], SDT, tag="vb")
                nc.gpsimd.dma_start(vb[:], v_c[:])
                v4 = v4_pool.tile([128, CQ], SDT, tag="v4")
                nc.gpsimd.dma_start(
                    v4[:], vb[:].unsqueeze(0).broadcast_to((4, 32, CQ)))

                ar_handles = []
                s_ps = psB.tile([32, CQ], F32, tag="sacc")
                for s in range(NSWEEP):
                    if s == SPLIT_SWEEP:
                        ar_handles.append(flush_and_allreduce(s_ps, f"r{r}a"))
                        s_ps = psB.tile([32, CQ], F32, tag="sacc")
                    first, last_s = (s == 0 or s == SPLIT_SWEEP), \
                        (s == SPLIT_SWEEP - 1 or s == NSWEEP - 1)
                    if last:
                        it = ihw_pool.tile([128, CQ], F32, tag="it_w")
                        nc.sync.dma_start(it[:], ih_d[2 * s:2 * (s + 1)])
                    else:
                        it = ih_pool.tile([128, CQ], BF16, tag="it_l")
                        nc.sync.dma_start(it[:], ihb_d[2 * s:2 * (s + 1)])
                    pr = prod_pool.tile(
                        [128, CQ], SDT, tag="pr_f" if last else "pr_b")
                    nc.vector.tensor_mul(pr[:], it[:], v4[:])
                    dlog = small_pool.tile([128, C], F32, tag="dlog")
                    nc.vector.reduce_sum(
                        dlog[:], pr[:].rearrange("p (c q) -> p c q", q=Q),
                        axis=AX.X)
                    bsl = b_acc[:, 32 * s:32 * (s + 1)]
                    nc.vector.tensor_add(bsl, bsl, dlog[:])
                    # softmax over c: coef = exp(b) / Z; 1/Z is folded into
                    # the selector matmul weights, exp broadcast to (c,q).
                    e = small_pool.tile([128, C], F32, tag="e")
                    nc.scalar.activation(e[:], bsl, AF.Exp)
                    z = small_pool.tile([128, 1], F32, tag="z")
                    nc.vector.reduce_sum(z[:], e[:], axis=AX.X)
                    rz = small_pool.tile([128, 1], F32, tag="rz")
                    nc.vector.reciprocal(rz[:], z[:])
                    selz = small_pool.tile([128, 32], SDT, tag="selz")
                    nc.vector.tensor_scalar_mul(
                        selz[:], sel_t[:] if last else sel_bf[:], rz[:])
                    e4 = wt_pool.tile([128, CQ], SDT, tag="e4")
                    nc.scalar.activation(
                        e4[:].rearrange("p (c q) -> p c q", q=Q),
                        e[:].unsqueeze(-1).broadcast_to((128, C, Q)), AF.Copy)
                    wtt = wt_pool.tile([128, CQ], SDT, tag="wtt")
                    nc.vector.tensor_mul(wtt[:], it[:], e4[:])
                    for h in range(4):
                        sl = slice(512 * h, 512 * (h + 1))
                        nc.tensor.matmul(
                            s_ps[:, sl], selz[:], wtt[:, sl],
                            start=first, stop=last_s)
                ar_handles.append(flush_and_allreduce(s_ps, f"r{r}b"))

    nc.compile()
    return nc


def _split_bf16(a):
    hi = a.astype(BF)
    lo = (a - hi.astype(np.float32)).astype(BF)
    return hi, lo


def _prep_core_inputs(x, W):
    """Host-side shard + repack for one call. Returns list of in_maps."""
    xs_all = np.ascontiguousarray(x)          # [B, I, P]
    in_maps = []
    sel = np.tile(np.eye(32, dtype=np.float32), (4, 1))
    for k in range(N_CORES):
        xs = xs_all[:, k * IL:(k + 1) * IL, :]          # [B, IL, P]
        # lhsT[t][i2*64+p, col]:
        #   cols 0:32   <- xs[b, 2t, p]        (block i2=0)
        #   cols 32:64  <- xs[b, 2t+1, p]      (block i2=1)
        #   cols 64:96  <- xs[b, 2t+i2, p]/32  (both blocks: s0 accumulation)
        xt = xs.transpose(1, 2, 0).reshape(NPAIR, 2, P, B)  # [t, i2, p, b]
        lhsT = np.zeros((NPAIR, 128, 96), np.float32)
        lhsT[:, 0:64, 0:32] = xt[:, 0]
        lhsT[:, 64:128, 32:64] = xt[:, 1]
        lhsT[:, 0:64, 64:96] = xt[:, 0] / C
        lhsT[:, 64:128, 64:96] = xt[:, 1] / C
        lhi, llo = _split_bf16(lhsT)
        if CONFIG["p1_single"]:
            lall = lhi                                   # [t, 128, 96]
        else:
            lall = np.concatenate([lhi, llo], axis=2)    # [t, 128, 192]
        lall = np.ascontiguousarray(lall.transpose(1, 0, 2)).reshape(128, -1)
        Ws = W[k * IL:(k + 1) * IL]                      # [IL, C, P, Q]
        w_rhs = np.ascontiguousarray(
            Ws.reshape(NPAIR, 2, C, P, Q).transpose(0, 1, 3, 2, 4)
        ).reshape(NPAIR, 128, CQ)
        whi, wlo = _split_bf16(w_rhs)
        if CONFIG["p1_single"]:
            w2 = np.ascontiguousarray(whi)               # [t, 128, 2048]
        else:
            w2 = np.concatenate([whi, wlo], axis=2)      # [t, 128, 4096]
        in_maps.append({"lhsT": lall, "w_rhs": w2, "sel": sel})
    return in_maps


def _host_reference(x, W, bias):
    """Exact numpy fallback (used only if bias != 0, which the problem's
    input spec says cannot happen; the device kernel assumes uniform
    round-0 routing coefficients)."""
    ih = np.einsum("bip,icpq->bicq", x, W)
    b = bias.astype(np.float64)
    out = None
    for r in range(NUM_ROUTING):
        e = np.exp(b - b.max(axis=2, keepdims=True))
        c = e / e.sum(axis=2, keepdims=True)
        s = (c * ih).sum(axis=1, keepdims=True)
        sq = np.sum(s * s, axis=-1, keepdims=True)
        out = s * (sq / (1.0 + sq) / np.sqrt(sq))
        if r != NUM_ROUTING - 1:
            b = b + np.sum(ih * out, axis=-1, keepdims=True)
    return out.reshape(B, C, Q).astype(np.float32)


def kernel(x, W, bias):
    global _compiled, _compiled_cfg
    x = np.asarray(x, dtype=np.float32)
    W = np.asarray(W, dtype=np.float32)
    bias = np.asarray(bias, dtype=np.float32)
    if np.any(bias):
        return _host_reference(x, W, bias)

    cfg = (CONFIG["r2_f32"], CONFIG["p1_single"])
    if _compiled is None or _compiled_cfg != cfg:
        _compiled = _build_kernel()
        _compiled_cfg = cfg
    nc = _compiled

    in_maps = _prep_core_inputs(x, W)
    res = run_bass_kernel_spmd(
        nc, in_maps, list(range(N_CORES)),
        trace=CONFIG["trace"], trace_cores=CONFIG["trace_cores"],
    )
    kernel.last_results = res
    out = res.results[0]["out"].reshape(B, C, Q)
    return out


# revision 36
# speedup vs baseline: 1.0298x; 1.0211x over previous
"""Trainium2 Bass kernel for CapsuleLayer (dynamic routing) on 8 NeuronCores.

Problem: x[32,1152,64], W[1152,32,64,64], bias[1,1152,32,1] (zeros) ->
         out[32,32,64]
  inputs_hat = einsum('bip,icpq->bicq', x, W)
  3 rounds of routing (softmax over capsule axis, squash, agreement update).

Sharding: input-capsule axis i=1152 split over 8 cores (144 each).
  - W read exactly once per core (75.5 MB), ih materialized to HBM per core.
  - softmax over output capsules c is core-local; only the [32,32,64]
    pre-squash sum is AllReduced per routing round (split into two
    AllReduces so the first overlaps the tail of the round's compute).

Phase 1 per core: 72 block-diagonal matmul pairs (2 i's packed -> K=128)
  compute ih[b,i,c,q]. x and W are split hi/lo into bf16 on the host;
  ih = x_hi@W_hi + x_hi@W_lo + x_lo@W_hi runs at full PE rate with ~1e-5
  relative error. Extra lhsT columns accumulate s0 = sum_i ih / 32 in PSUM
  (bias==0 so round-0 routing coefficients are uniform 1/32). ih is stored
  to HBM in bf16 (round-1 logit pass) and fp32 (final round).

Routing rounds r=1,2: one fused pass per 4-i sweep over ih:
  logits += <ih, v> (DVE mult + reduce over q), softmax over c via exp on
  ScalarE broadcast to the (c,q) grid, s += exp * ih with the 1/Z softmax
  denominator folded into the selector matmul weights (PSUM accumulation).
  Round 1 runs in bf16 (it only steers coefficients); the final round runs
  fp32 end-to-end from the fp32 ih copy.
"""

import os
import sys

import numpy as np

for _p in (
    "/opt/trn_rl_repo",
    "/root/.axon_site",
    "/root/.axon_site/_ro/trn_rl_repo",
    "/root/.axon_site/_ro/pypackages",
):
    if os.path.isdir(_p) and _p not in sys.path:
        sys.path.append(_p)

import ml_dtypes
import concourse.bacc as bacc
import concourse.mybir as mybir
import concourse.tile as tile
from concourse.bass_utils import run_bass_kernel_spmd

F32 = mybir.dt.float32
BF16 = mybir.dt.bfloat16
AF = mybir.ActivationFunctionType
AX = mybir.AxisListType
ALU = mybir.AluOpType
BF = ml_dtypes.bfloat16

B, I, P, C, Q = 32, 1152, 64, 32, 64
N_CORES = 8
IL = I // N_CORES          # 144 input capsules per core
NPAIR = IL // 2            # 72 matmul pairs
NSWEEP = IL // 4           # 36 routing sweeps (4 i's x 32 b = 128 partitions)
CQ = C * Q                 # 2048
NUM_ROUTING = 3
SPLIT_PAIR = 56            # s0 partial AllReduced after this many pairs
SPLIT_SWEEP = 30           # routing partial AllReduced after this many sweeps

CONFIG = {
    "r2_f32": False,          # final round weighted sum in fp32 (else bf16)
    "p1_single": True,        # phase-1 matmul = x_hi @ W_hi only (skip the
                              # hi/lo correction terms: ih is consumed through
                              # its bf16 copy anyway)
    "trace": False,           # profile the run (exec_time_ns); needs ntff hook
    "trace_cores": None,      # None -> core 0 only
}

_compiled = None
_compiled_cfg = None


def _build_kernel():
    """Build + compile the SPMD Bass module (identical program on 8 cores)."""
    nc = bacc.Bacc("TRN2", target_bir_lowering=False, debug=False,
                   num_devices=N_CORES)

    p1s = CONFIG["p1_single"]
    lstride = 96 if p1s else 192
    # lhsT_all[p, t*stride + (0:96)] = hi half, (96:192) = lo half for pair t
    lall_d = nc.dram_tensor("lhsT", [128, NPAIR * lstride], BF16,
                            kind="ExternalInput")
    # w2[t, p, 0:2048] = hi, [t, p, 2048:4096] = lo (lo absent if p1_single)
    w_d = nc.dram_tensor("w_rhs", [NPAIR, 128, CQ if p1s else 2 * CQ], BF16,
                         kind="ExternalInput")
    sel_d = nc.dram_tensor("sel", [128, 32], F32, kind="ExternalInput")
    out_d = nc.dram_tensor("out", [B, CQ], F32, kind="ExternalOutput")

    r2f = CONFIG["r2_f32"]
    assert not (r2f and CONFIG["p1_single"]), \
        "r2_f32 needs the fp32 ih copy from the multi-term phase 1"
    ih_d = nc.dram_tensor("ih_buf", [NPAIR, 64, CQ], F32) if r2f else None
    ihb_d = nc.dram_tensor("ihb_buf", [NPAIR, 64, CQ], BF16)

    rgroups = [list(range(N_CORES))]

    with tile.TileContext(nc) as tc:
        with (
            tc.tile_pool(name="lall", bufs=1) as lall_pool,
            tc.tile_pool(name="w", bufs=5) as w_pool,
            tc.tile_pool(name="cp", bufs=3) as cp_pool,
            tc.tile_pool(name="ih", bufs=8) as ih_pool,
            tc.tile_pool(name="ihw", bufs=2) as ihw_pool,
            tc.tile_pool(name="prod", bufs=4) as prod_pool,
            tc.tile_pool(name="wt", bufs=4) as wt_pool,
            tc.tile_pool(name="v4", bufs=1) as v4_pool,
            tc.tile_pool(name="small", bufs=3) as small_pool,
            tc.tile_pool(name="acc", bufs=1) as acc_pool,
            tc.tile_pool(name="sv", bufs=1) as sv_pool,
            tc.tile_pool(name="psA", bufs=4, space="PSUM") as psA,
            tc.tile_pool(name="psB", bufs=1, space="PSUM") as psB,
            tc.tile_pool(name="dram", bufs=2, space="DRAM") as dram_pool,
        ):
            sel_t = small_pool.tile([128, 32], F32, tag="sel")
            nc.sync.dma_start(sel_t[:], sel_d[:])
            sel_bf = small_pool.tile([128, 32], BF16, tag="selbf")
            nc.vector.tensor_copy(sel_bf[:], sel_t[:])

            b_acc = acc_pool.tile([128, NSWEEP * 32], F32, tag="bacc")
            nc.vector.memset(b_acc[:], 0.0)

            lall = lall_pool.tile([128, NPAIR * lstride], BF16, tag="lall")
            nc.sync.dma_start(lall[:], lall_d[:])

            # Warm-up AllReduce: the first collective pays a large one-time
            # staging/rendezvous cost; burn it here, hidden under phase 1.
            wu_sb = small_pool.tile([32, 16], F32, tag="wu")
            nc.vector.memset(wu_sb[:], 0.0)
            wu_in = dram_pool.tile([32, 16], F32, tag="wu_in")
            wu_out = dram_pool.tile([32, 16], F32, tag="wu_out")
            nc.gpsimd.dma_start(wu_in[:], wu_sb[:])
            nc.gpsimd.collective_compute(
                "AllReduce", ALU.add,
                ins=[wu_in[:].opt()], outs=[wu_out[:].opt()],
                replica_groups=rgroups,
            )

            def flush_and_allreduce(s_ps, tag):
                """PSUM partial -> SBUF -> DRAM -> AllReduce. Returns the
                collective's DRAM output tile."""
                f_sb = sv_pool.tile([32, CQ], F32, tag="s_sb")
                nc.scalar.copy(f_sb[:], s_ps[:])
                a_in = dram_pool.tile([32, CQ], F32, tag="ar_in")
                a_out = dram_pool.tile([32, CQ], F32, tag="ar_out")
                nc.gpsimd.dma_start(a_in[:], f_sb[:])
                nc.gpsimd.collective_compute(
                    "AllReduce", ALU.add,
                    ins=[a_in[:].opt()], outs=[a_out[:].opt()],
                    replica_groups=rgroups,
                )
                return a_out

            # ---------------- Phase 1: ih = x @ W, s0 = sum_i ih / 32 -------
            s_ps = psB.tile([32, CQ], F32, tag="sacc")
            ar_handles = []
            for t in range(NPAIR):
                if t == SPLIT_PAIR:
                    ar_handles.append(flush_and_allreduce(s_ps, "s0a"))
                    s_ps = psB.tile([32, CQ], F32, tag="sacc")
                first, last_t = (t == 0 or t == SPLIT_PAIR), \
                    (t == SPLIT_PAIR - 1 or t == NPAIR - 1)
                lhi = lall[:, t * lstride:t * lstride + 96]
                wt_ = w_pool.tile([128, CQ if p1s else 2 * CQ], BF16)
                nc.sync.dma_start(wt_[:], w_d[t])
                cp = None
                if not p1s:
                    llo = lall[:, t * lstride + 96:t * lstride + 192]
                    cp = cp_pool.tile([64, CQ], F32, tag="cp32")
                cpb = cp_pool.tile([64, CQ], BF16, tag="cpb")
                if p1s:
                    # group same-lhsT matmuls so LDWEIGHTS doesn't alternate
                    pss = []
                    for blk in range(4):
                        sl = slice(512 * blk, 512 * (blk + 1))
                        ps = psA.tile([64, 512], F32)
                        pss.append(ps)
                        nc.tensor.matmul(ps[:], lhi[:, 0:64], wt_[:, sl],
                                         start=True, stop=True)
                    for blk in range(4):
                        sl = slice(512 * blk, 512 * (blk + 1))
                        nc.tensor.matmul(
                            s_ps[:, sl], lhi[:, 64:96], wt_[:, sl],
                            start=first, stop=last_t,
                        )
                        # alternate ACT/DVE so neither becomes the pipeline
                        # stage bound (DVE is otherwise idle in phase 1)
                        if blk % 2 == 0:
                            nc.scalar.copy(cpb[:, sl], pss[blk][:])
                        else:
                            nc.vector.tensor_copy(cpb[:, sl], pss[blk][:])
                else:
                    for blk in range(4):
                        sl = slice(512 * blk, 512 * (blk + 1))
                        ps = psA.tile([64, 512], F32)
                        whi = wt_[:, sl]
                        wlo = wt_[:, CQ + 512 * blk:CQ + 512 * (blk + 1)]
                        nc.tensor.matmul(ps[:], lhi[:, 0:64], whi,
                                         start=True, stop=False)
                        nc.tensor.matmul(ps[:], lhi[:, 0:64], wlo,
                                         start=False, stop=False)
                        nc.tensor.matmul(ps[:], llo[:, 0:64], whi,
                                         start=False, stop=True)
                        nc.tensor.matmul(
                            s_ps[:, sl], lhi[:, 64:96], whi,
                            start=first, stop=last_t,
                        )
                        nc.scalar.copy(cp[:, sl], ps[:])
                if not p1s:
                    nc.vector.tensor_copy(cpb[:], cp[:])
                    if r2f:
                        nc.sync.dma_start(ih_d[t], cp[:])
                nc.sync.dma_start(ihb_d[t], cpb[:])
            ar_handles.append(flush_and_allreduce(s_ps, "s0b"))

            # ---------------- Routing rounds -------------------------------
            # The [32, b*(c,q)] sum is reshaped to [128, 512] (partition =
            # (b, cq-quarter)) so the squash math runs on all 128 lanes.
            CQ4 = CQ // 4   # 512
            C4 = C // 4     # 8 capsules per partition-quarter
            for r in range(1, NUM_ROUTING + 1):
                # combine the two AllReduce halves, then squash -> v
                pa = sv_pool.tile([128, CQ4], F32, tag="pa")
                pb = sv_pool.tile([128, CQ4], F32, tag="pb")
                nc.gpsimd.dma_start(pa[:], ar_handles[0][:])
                nc.gpsimd.dma_start(pb[:], ar_handles[1][:])
                S_sb = sv_pool.tile([128, CQ4], F32, tag="S_sb")
                nc.vector.tensor_add(S_sb[:], pa[:], pb[:])

                # squash: v = S * sqrt(sq)/(1+sq),  sq = sum_q S^2
                S3 = S_sb[:].rearrange("b (c q) -> b c q", q=Q)
                sq = small_pool.tile([128, C4], F32, tag="sq")
                sqr = sv_pool.tile([128, CQ4], F32, tag="sqr")
                nc.vector.tensor_mul(sqr[:], S_sb[:], S_sb[:])
                nc.vector.reduce_sum(
                    sq[:], sqr[:].rearrange("b (c q) -> b c q", q=Q),
                    axis=AX.X)
                rt = small_pool.tile([128, C4], F32, tag="rt")
                nc.scalar.sqrt(rt[:], sq[:])
                onep = small_pool.tile([128, C4], F32, tag="onep")
                nc.vector.tensor_scalar_add(onep[:], sq[:], 1.0)
                rden = small_pool.tile([128, C4], F32, tag="rden")
                nc.vector.reciprocal(rden[:], onep[:])
                scale = small_pool.tile([128, C4], F32, tag="scale")
                nc.vector.tensor_mul(scale[:], rt[:], rden[:])
                # v overwrites S_sb in place
                nc.vector.tensor_mul(
                    S3, S3, scale[:].unsqueeze(-1).broadcast_to((128, C4, Q)))
                v_t = S_sb

                if r == NUM_ROUTING:
                    nc.sync.dma_start(out_d[:], v_t[:])
                    break

                last = (r == NUM_ROUTING - 1) and r2f
                SDT = F32 if last else BF16   # sweep compute dtype

                # broadcast v across the 4 i-slots of the 128 partitions
                # (DRAM bounce; the stride-0 broadcast read is legal there)
                v_c = sv_pool.tile([128, CQ4], SDT, tag="v_c")
                nc.vector.tensor_copy(v_c[:], v_t[:])
                vb = dram_pool.tile([32, CQ], SDT, tag="vb")
                nc.gpsimd.dma_start(vb[:], v_c[:])
                v4 = v4_pool.tile([128, CQ], SDT, tag="v4")
                nc.gpsimd.dma_start(
                    v4[:], vb[:].unsqueeze(0).broadcast_to((4, 32, CQ)))

                ar_handles = []
                s_ps = psB.tile([32, CQ], F32, tag="sacc")
                for s in range(NSWEEP):
                    if s == SPLIT_SWEEP:
                        ar_handles.append(flush_and_allreduce(s_ps, f"r{r}a"))
                        s_ps = psB.tile([32, CQ], F32, tag="sacc")
                    first, last_s = (s == 0 or s == SPLIT_SWEEP), \
                        (s == SPLIT_SWEEP - 1 or s == NSWEEP - 1)
                    if last:
                        it = ihw_pool.tile([128, CQ], F32, tag="it_w")
                        nc.sync.dma_start(it[:], ih_d[2 * s:2 * (s + 1)])
                    else:
                        it = ih_pool.tile([128, CQ], BF16, tag="it_l")
                        nc.sync.dma_start(it[:], ihb_d[2 * s:2 * (s + 1)])
                    pr = prod_pool.tile(
                        [128, CQ], SDT, tag="pr_f" if last else "pr_b")
                    nc.vector.tensor_mul(pr[:], it[:], v4[:])
                    dlog = small_pool.tile([128, C], F32, tag="dlog")
                    nc.vector.reduce_sum(
                        dlog[:], pr[:].rearrange("p (c q) -> p c q", q=Q),
                        axis=AX.X)
                    bsl = b_acc[:, 32 * s:32 * (s + 1)]
                    nc.vector.tensor_add(bsl, bsl, dlog[:])
                    # softmax over c: coef = exp(b) / Z; 1/Z is folded into
                    # the selector matmul weights, exp broadcast to (c,q).
                    e = small_pool.tile([128, C], F32, tag="e")
                    nc.scalar.activation(e[:], bsl, AF.Exp)
                    z = small_pool.tile([128, 1], F32, tag="z")
                    nc.vector.reduce_sum(z[:], e[:], axis=AX.X)
                    rz = small_pool.tile([128, 1], F32, tag="rz")
                    nc.vector.reciprocal(rz[:], z[:])
                    selz = small_pool.tile([128, 32], SDT, tag="selz")
                    nc.vector.tensor_scalar_mul(
                        selz[:], sel_t[:] if last else sel_bf[:], rz[:])
                    e4 = wt_pool.tile([128, CQ], SDT, tag="e4")
                    nc.scalar.activation(
                        e4[:].rearrange("p (c q) -> p c q", q=Q),
                        e[:].unsqueeze(-1).broadcast_to((128, C, Q)), AF.Copy)
                    wtt = wt_pool.tile([128, CQ], SDT, tag="wtt")
                    nc.vector.tensor_mul(wtt[:], it[:], e4[:])
                    for h in range(4):
                        sl = slice(512 * h, 512 * (h + 1))
                        nc.tensor.matmul(
                            s_ps[:, sl], selz[:], wtt[:, sl],
                            start=first, stop=last_s)
                ar_handles.append(flush_and_allreduce(s_ps, f"r{r}b"))

    nc.compile()
    return nc


def _split_bf16(a):
    hi = a.astype(BF)
    lo = (a - hi.astype(np.float32)).astype(BF)
    return hi, lo


def _prep_core_inputs(x, W):
    """Host-side shard + repack for one call. Returns list of in_maps."""
    xs_all = np.ascontiguousarray(x)          # [B, I, P]
    in_maps = []
    sel = np.tile(np.eye(32, dtype=np.float32), (4, 1))
    for k in range(N_CORES):
        xs = xs_all[:, k * IL:(k + 1) * IL, :]          # [B, IL, P]
        # lhsT[t][i2*64+p, col]:
        #   cols 0:32   <- xs[b, 2t, p]        (block i2=0)
        #   cols 32:64  <- xs[b, 2t+1, p]      (block i2=1)
        #   cols 64:96  <- xs[b, 2t+i2, p]/32  (both blocks: s0 accumulation)
        xt = xs.transpose(1, 2, 0).reshape(NPAIR, 2, P, B)  # [t, i2, p, b]
        lhsT = np.zeros((NPAIR, 128, 96), np.float32)
        lhsT[:, 0:64, 0:32] = xt[:, 0]
        lhsT[:, 64:128, 32:64] = xt[:, 1]
        lhsT[:, 0:64, 64:96] = xt[:, 0] / C
        lhsT[:, 64:128, 64:96] = xt[:, 1] / C
        lhi, llo = _split_bf16(lhsT)
        if CONFIG["p1_single"]:
            lall = lhi                                   # [t, 128, 96]
        else:
            lall = np.concatenate([lhi, llo], axis=2)    # [t, 128, 192]
        lall = np.ascontiguousarray(lall.transpose(1, 0, 2)).reshape(128, -1)
        Ws = W[k * IL:(k + 1) * IL]                      # [IL, C, P, Q]
        w_rhs = np.ascontiguousarray(
            Ws.reshape(NPAIR, 2, C, P, Q).transpose(0, 1, 3, 2, 4)
        ).reshape(NPAIR, 128, CQ)
        whi, wlo = _split_bf16(w_rhs)
        if CONFIG["p1_single"]:
            w2 = np.ascontiguousarray(whi)               # [t, 128, 2048]
        else:
            w2 = np.concatenate([whi, wlo], axis=2)      # [t, 128, 4096]
        in_maps.append({"lhsT": lall, "w_rhs": w2, "sel": sel})
    return in_maps


def _host_reference(x, W, bias):
    """Exact numpy fallback (used only if bias != 0, which the problem's
    input spec says cannot happen; the device kernel assumes uniform
    round-0 routing coefficients)."""
    ih = np.einsum("bip,icpq->bicq", x, W)
    b = bias.astype(np.float64)
    out = None
    for r in range(NUM_ROUTING):
        e = np.exp(b - b.max(axis=2, keepdims=True))
        c = e / e.sum(axis=2, keepdims=True)
        s = (c * ih).sum(axis=1, keepdims=True)
        sq = np.sum(s * s, axis=-1, keepdims=True)
        out = s * (sq / (1.0 + sq) / np.sqrt(sq))
        if r != NUM_ROUTING - 1:
            b = b + np.sum(ih * out, axis=-1, keepdims=True)
    return out.reshape(B, C, Q).astype(np.float32)


def kernel(x, W, bias):
    global _compiled, _compiled_cfg
    x = np.asarray(x, dtype=np.float32)
    W = np.asarray(W, dtype=np.float32)
    bias = np.asarray(bias, dtype=np.float32)
    if np.any(bias):
        return _host_reference(x, W, bias)

    cfg = (CONFIG["r2_f32"], CONFIG["p1_single"])
    if _compiled is None or _compiled_cfg != cfg:
        _compiled = _build_kernel()
        _compiled_cfg = cfg
    nc = _compiled

    in_maps = _prep_core_inputs(x, W)
    res = run_bass_kernel_spmd(
        nc, in_maps, list(range(N_CORES)),
        trace=CONFIG["trace"], trace_cores=CONFIG["trace_cores"],
    )
    kernel.last_results = res
    out = res.results[0]["out"].reshape(B, C, Q)
    return out
